# revision 16
# baseline (speedup 1.0000x reference)
"""Trainium2 Bass kernel for nn_Decoder_19774029431790.

Pure data-parallel over batch B=8 -> 8 NeuronCores, one sample per core.

Per-core pipeline (layout: channels on partitions, pixels on free dim):
  1. 1x1 conv 888->64 as PE matmul (K tiled 7x128), with an extra "mean"
     output row fused in (lhsT column of 1/888) and output columns
     pre-permuted into channel-shuffle order.
  2. channel max (pad2) via free-dim folds + gpsimd partition_all_reduce.
  3. three grouped depthwise branch convs (3x3/5x5/7x7) as one union 7x7
     depthwise pass over 66 channels; taps split across PE (diagonal
     matmuls accumulating in PSUM), DVE and Pool (scalar_tensor_tensor).
  4. Wpe depthwise 3x3 with identity folded into the center tap.
  5. LayerNorm over channels: sums via ones-matmul, per-pixel stats on one
     partition, broadcast back via gpsimd partition_broadcast.
  6. Wlast dense 3x3 conv as 9 shifted PSUM-accumulated matmuls; Gelu+bias
     on ACT; residual add of the (gapped) x64.
  7. aux head: depthwise 3x3, depthwise 5x5 dil3, 1x1 to 6 channels.
  8. bilinear 32x32 -> 1024x1024 resize as two matmul expansions with the
     interpolation matrix A (exactly matches jax.image.resize).
     Output streamed PSUM -> SBUF (DVE/ACT alternating) -> HBM in 2MB DMAs.

The channel "O-order" used on-chip after the shuffle has a dead row at
partition 43 (so PSUM->SBUF copies never need mismatched partition bases);
all weights touching that row are zeroed host-side.
"""

import os
from contextlib import ExitStack

import numpy as np

import concourse.bass as bass
import concourse.bacc as bacc
import concourse.bass_isa as bass_isa
import concourse.mybir as mybir
import concourse.tile as tile
from concourse.bass_utils import run_bass_kernel_spmd

F32 = mybir.dt.float32
F32R = mybir.dt.float32r
AF = mybir.ActivationFunctionType
OP = mybir.AluOpType

NCORES = 8
CIN = 888
E = 66  # post-fusion channels
C3 = 22
D = 64
NCH_OUT = 6
HW = 1024  # 32*32
HO = 1024

USE_F32R = os.environ.get("BASS_NO_F32R", "0") != "1"
RD = F32R if USE_F32R else F32  # dtype for tensors consumed by fast fp32 matmuls

# channel-shuffle source order: shuffled[j] = x_f[src], x_f = [x64(64), pad1, pad2]
# O-order (64 entries) = sources of shuffled channels that come from x64
O_PERM = [3 * c for c in range(22)] + [3 * c + 1 for c in range(21)] + [3 * c + 2 for c in range(21)]
# 65-row gapped layout: row 43 dead (pad1 slot), rows 44..64 = O_PERM[43:]
ROW65 = O_PERM[:43] + [None] + O_PERM[43:]

# depthwise tap splits per stage: (n_pe, n_dve, n_pool)
SPLIT = {
    "branch": (34, 15, 0),   # 49 taps
    "wpe": (6, 3, 0),        # 9 taps
    "a0": (6, 3, 0),         # 9 taps
    "was": (17, 8, 0),       # 25 taps
}

BR_TAPS = [(dy, dx) for dy in range(-3, 4) for dx in range(-3, 4)]
PE_TAPS9 = [(dy, dx) for dy in range(-1, 2) for dx in range(-1, 2)]
WAS_TAPS = [(dy, dx) for dy in (-6, -3, 0, 3, 6) for dx in (-6, -3, 0, 3, 6)]


def bilinear_matrix(n_in: int, n_out: int) -> np.ndarray:
    """Row-stochastic interpolation matrix matching jax.image.resize bilinear."""
    scale = n_in / n_out
    x = (np.arange(n_out) + 0.5) * scale - 0.5
    j = np.arange(n_in)
    w = np.maximum(0.0, 1.0 - np.abs(x[:, None] - j[None, :]))
    w = w / w.sum(1, keepdims=True)
    return np.ascontiguousarray(w, np.float32)


def _gap65(arr64: np.ndarray, fill: float = 0.0) -> np.ndarray:
    """[64, ...] in original channel order -> [65, ...] in gapped O-order."""
    out = np.full((65,) + arr64.shape[1:], fill, arr64.dtype)
    for i, src in enumerate(ROW65):
        if src is not None:
            out[i] = arr64[src]
    return out


def host_consts(inp: dict) -> dict[str, np.ndarray]:
    f = lambda a: np.asarray(a, np.float32)
    W1, b1 = f(inp["W_1x1"])[:, :, 0, 0], f(inp["b_1x1"])  # [64,888],[64]
    W3, b3 = f(inp["W3"])[:, 0], f(inp["b3"])
    W5, b5 = f(inp["W5"])[:, 0], f(inp["b5"])
    W7, b7 = f(inp["W7"])[:, 0], f(inp["b7"])
    Wpe, bpe = f(inp["Wpe"])[:, 0], f(inp["bpe"])
    ln_g, ln_b = f(inp["ln_g"]), f(inp["ln_b"])
    Wlast, blast = f(inp["Wlast"]), f(inp["blast"])  # [64,66,3,3],[64]
    Wa0 = f(inp["Wa0"])[:, 0]  # [64,3,3]
    Was = f(inp["Was"])[:, 0]  # [64,5,5]
    Wout = f(inp["Wout"])[:, :, 0, 0]  # [6,64]

    # --- 1x1 conv lhsT, K padded 888->896, with fused mean row and O-gapped cols
    Wq = np.zeros((896, 66), np.float32)
    for j in range(65):
        if j == 43:
            Wq[:CIN, j] = 1.0 / CIN
        else:
            Wq[:CIN, j] = W1[ROW65[j]]
    # column 65 stays zero: xs row 65 (pad2/channel-max) is added separately
    wmm = np.ascontiguousarray(Wq.reshape(7, 128, 66).transpose(1, 0, 2))  # [128,7,66]

    b65 = np.zeros(65, np.float32)
    for j in range(65):
        if j != 43:
            b65[j] = b1[ROW65[j]]

    # --- branch union taps [49, 66] on shuffled channels
    wt_br = np.zeros((49, 66), np.float32)
    for k, (dy, dx) in enumerate(BR_TAPS):
        for j in range(66):
            br, c = j // 22, j % 22
            if br == 0 and abs(dy) <= 1 and abs(dx) <= 1:
                wt_br[k, j] = W3[c, dy + 1, dx + 1]
            elif br == 1 and abs(dy) <= 2 and abs(dx) <= 2:
                wt_br[k, j] = W5[c, dy + 2, dx + 2]
            elif br == 2:
                wt_br[k, j] = W7[c, dy + 3, dx + 3]
    b66 = np.concatenate([b3, b5, b7]).astype(np.float32)

    # --- Wpe taps [9, 66], identity folded into center
    wt_pe = np.zeros((9, 66), np.float32)
    for k, (dy, dx) in enumerate(PE_TAPS9):
        wt_pe[k] = Wpe[:, dy + 1, dx + 1]
        if dy == 0 and dx == 0:
            wt_pe[k] += 1.0
    # --- a0 / was taps on gapped 65 rows
    wt_a0 = np.stack([_gap65(Wa0[:, dy + 1, dx + 1]) for (dy, dx) in PE_TAPS9])  # [9,65]
    wt_was = np.stack(
        [_gap65(Was[:, dy // 3 + 2, dx // 3 + 2]) for (dy, dx) in WAS_TAPS]
    )  # [25,65]

    blast65 = _gap65(blast)
    woutT = _gap65(Wout.T.copy().reshape(64, 6))  # [65,6]

    # --- const tables (channels on partitions)
    # ctab66 cols: 49 branch taps | 9 pe taps | b66 | b65(pad row65=0) | ln_g | ln_b | ones/66 | bpe
    ctab66 = np.zeros((66, 49 + 9 + 7), np.float32)
    ctab66[:, 0:49] = wt_br.T
    ctab66[:, 49:58] = wt_pe.T
    ctab66[:, 58] = b66
    ctab66[:65, 59] = b65
    ctab66[:, 60] = ln_g
    ctab66[:, 61] = ln_b
    ctab66[:, 62] = 1.0 / 66.0
    ctab66[:, 63] = bpe
    ctab66[65, 64] = 1.0  # pad2 row mask
    # ctab65 cols: 9 a0 taps | 25 was taps | blast65 | 6 woutT cols
    ctab65 = np.zeros((65, 9 + 25 + 1 + 6), np.float32)
    ctab65[:, 0:9] = wt_a0.T
    ctab65[:, 9:34] = wt_was.T
    ctab65[:, 34] = blast65
    ctab65[:, 35:41] = woutT

    # --- Wlast lhsT per tap: [66, 9, 65]
    wl = np.zeros((66, 9, 65), np.float32)
    for t, (dy, dx) in enumerate(PE_TAPS9):
        for i in range(65):
            if ROW65[i] is not None:
                wl[:, t, i] = Wlast[ROW65[i], :, dy + 1, dx + 1]

    # --- PE-tap diagonal weight matrices
    npe_br, _, _ = SPLIT["branch"]
    npe_pe, _, _ = SPLIT["wpe"]
    npe_a0, _, _ = SPLIT["a0"]
    npe_was, _, _ = SPLIT["was"]
    d66 = np.zeros((66, (npe_br + npe_pe) * 66), np.float32)
    for i in range(npe_br):
        d66[:, i * 66 : (i + 1) * 66] = np.diag(wt_br[i])
    for i in range(npe_pe):
        c0 = (npe_br + i) * 66
        d66[:, c0 : c0 + 66] = np.diag(wt_pe[i])
    d65 = np.zeros((65, (npe_a0 + npe_was) * 65), np.float32)
    for i in range(npe_a0):
        d65[:, i * 65 : (i + 1) * 65] = np.diag(wt_a0[i])
    for i in range(npe_was):
        c0 = (npe_a0 + i) * 65
        d65[:, c0 : c0 + 65] = np.diag(wt_was[i])

    amat = np.ascontiguousarray(bilinear_matrix(32, HO).T)  # [32, 1024] A.T
    zz = np.zeros((128, 1936), np.float32)  # zero-fill source for f32r tiles
    negf = np.full((32, 1024), -1e30, np.float32)  # pad fill for the channel-max

    return dict(
        zz=zz,
        negf=negf,
        wmm=wmm,
        ctab66=ctab66,
        ctab65=ctab65,
        wl=wl,
        diag66=d66,
        diag65=d65,
        amat=amat,
    )


def _mm(nc, out, lhsT, rhs, start=True, stop=True):
    nc.tensor.matmul(out, lhsT, rhs, start=start, stop=stop)


def _dw_stage(nc, sb, ps, name, taps, split, src_fn, out_int, nch, wcol_fn, diag_ap_fn, bias_col):
    """Depthwise conv stage: out_int (strided/flat [nch,32,32] view) = sum of taps.

    taps assigned: PE (diag matmuls into PSUM), then DVE, then POOL
    (each scalar_tensor_tensor accumulating in its own buffer), combined at end.
    """
    n_pe, n_dve, n_pool = split
    assert n_pe + n_dve + n_pool == len(taps) and n_dve >= 1
    pe_taps = list(range(n_pe))
    dve_taps = list(range(n_pe, n_pe + n_dve))
    pool_taps = list(range(n_pe + n_dve, len(taps)))

    # DVE chain (first op writes + carries bias)
    k = dve_taps[0]
    if bias_col is not None:
        nc.vector.tensor_scalar(out_int, src_fn(*taps[k]), wcol_fn(k), bias_col, OP.mult, OP.add)
    else:
        nc.vector.tensor_scalar_mul(out_int, src_fn(*taps[k]), wcol_fn(k))
    for k in dve_taps[1:]:
        nc.vector.scalar_tensor_tensor(out_int, src_fn(*taps[k]), wcol_fn(k), out_int, OP.mult, OP.add)

    # POOL chain
    xb = None
    if pool_taps:
        xb = sb.tile([nch, 32, 32], F32, name=f"xb_{name}", tag=f"xb_{name}")
        k = pool_taps[0]
        nc.gpsimd.tensor_scalar_mul(xb, src_fn(*taps[k]), wcol_fn(k))
        for k in pool_taps[1:]:
            nc.gpsimd.scalar_tensor_tensor(xb, src_fn(*taps[k]), wcol_fn(k), xb, OP.mult, OP.add)

    # PE chain
    pp = None
    if pe_taps:
        pp = ps.tile([nch, HW], F32, name=f"pp_{name}", tag="s")
        for i, k in enumerate(pe_taps):
            dy, dx = taps[k]
            src = src_fn(dy, dx)
            for h in (0, 1):
                _mm(
                    nc,
                    pp[:, h * 512 : (h + 1) * 512],
                    diag_ap_fn(i),
                    src[:, h * 16 : (h + 1) * 16, :],
                    start=(i == 0),
                    stop=(i == len(pe_taps) - 1),
                )

    # combine
    if pp is not None:
        ppv = pp.rearrange("c (h w) -> c h w", h=32)
        nc.vector.tensor_add(out_int, out_int, ppv)
    if xb is not None:
        nc.vector.tensor_add(out_int, out_int, xb)


def build_kernel_body(tc, out_ap, ins):
    """ins: dict name -> dram AP. out_ap: [6, 1024, 1024] dram AP."""
    nc = tc.nc
    ctx = ExitStack()
    sb = ctx.enter_context(tc.tile_pool(name="sb", bufs=1))
    ps = ctx.enter_context(tc.tile_pool(name="ps", bufs=2, space="PSUM"))
    psS = ctx.enter_context(tc.tile_pool(name="psS", bufs=1, space="PSUM"))
    sbD = ctx.enter_context(tc.tile_pool(name="sbD", bufs=2))

    n_br = SPLIT["branch"][0]
    n_wpe = SPLIT["wpe"][0]
    n_a0 = SPLIT["a0"][0]

    # ---------------- consts + input loads ----------------
    wmm = sb.tile([128, 7, 66], RD)
    nc.sync.dma_start(out=wmm, in_=ins["wmm"])
    ctab66 = sb.tile([66, 65], F32)
    nc.sync.dma_start(out=ctab66, in_=ins["ctab66"])
    ctab65 = sb.tile([65, 41], F32)
    nc.sync.dma_start(out=ctab65, in_=ins["ctab65"])
    wl = sb.tile([66, 9, 65], RD)
    nc.sync.dma_start(out=wl, in_=ins["wl"])
    amat = sb.tile([32, 1024], RD)
    nc.sync.dma_start(out=amat, in_=ins["amat"])
    diag66 = None
    if ins["diag66"].shape[1]:
        diag66 = sb.tile([66, ins["diag66"].shape[1]], RD)
        nc.sync.dma_start(out=diag66, in_=ins["diag66"])
    diag65 = None
    if ins["diag65"].shape[1]:
        diag65 = sb.tile([65, ins["diag65"].shape[1]], RD)
        nc.sync.dma_start(out=diag65, in_=ins["diag65"])

    xin = sb.tile([128, 7, HW], RD)
    x_flat = ins["x"].rearrange("c h w -> c (h w)")
    for k in range(6):
        nc.sync.dma_start(out=xin[:, k, :], in_=x_flat[k * 128 : (k + 1) * 128, :])
    nc.sync.dma_start(out=xin[96:128, 6, :], in_=ins["negf"])
    nc.sync.dma_start(out=xin[0:120, 6, :], in_=x_flat[768:888, :])

    wt_br = lambda k: ctab66[:, k : k + 1]
    wt_pe = lambda k: ctab66[:, 49 + k : 50 + k]
    b66_ap = ctab66[:, 58:59]
    b65_ap = ctab66[0:65, 59:60]
    g_ap = ctab66[:, 60:61]
    lb_ap = ctab66[:, 61:62]
    ones66 = ctab66[:, 62:63]
    bpe_ap = ctab66[:, 63:64]
    mask2_ap = ctab66[64:66, 64:65]
    wt_a0 = lambda k: ctab65[:, k : k + 1]
    wt_was = lambda k: ctab65[:, 9 + k : 10 + k]
    blast_ap = ctab65[:, 34:35]
    woutT_ap = ctab65[:, 35:41]

    # ---------------- stage buffers ----------------
    xs = sb.tile([66, 38, 38], RD)  # shuffled input, border 3
    xf = sb.tile([66, 34, 34], RD)  # branch output, border 1
    xf2 = sb.tile([66, 2048], F32)  # [wpe-out | squared]
    x64b = sb.tile([65, 32, 32], F32)
    lnp = sb.tile([66, 34, 34], RD)  # LN output, border 1
    outb = sb.tile([65, 34, 34], RD)  # gelu+resid, border 1
    a0 = sb.tile([65, 44, 44], RD)  # border 6
    w5t = sb.tile([65, 32, 32], F32)
    fts = sb.tile([32, 6, 32], RD)
    Ts = sb.tile([32, 6144], RD)
    acc = sb.tile([128, HW], F32)
    prd = sb.tile([128, HW], F32)
    epsv = sb.tile([1, 1], F32)
    nc.vector.memset(epsv, 1e-5)
    ms = sb.tile([1, 2048], F32)
    msq = sb.tile([1, 1024], F32)
    var = sb.tile([1, 1024], F32)
    stdv = sb.tile([1, 1024], F32)
    rmt = sb.tile([1, 2048], F32)
    bcast = sb.tile([66, 2048], F32)
    u66 = sb.tile([66, 1024], F32)

    zsrc = ins["zz"]
    nc.sync.dma_start(out=xs, in_=zsrc[0:66, 0 : 38 * 38].rearrange("c (h w) -> c h w", h=38))
    nc.sync.dma_start(out=xf, in_=zsrc[0:66, 0 : 34 * 34].rearrange("c (h w) -> c h w", h=34))
    nc.sync.dma_start(out=lnp, in_=zsrc[0:66, 0 : 34 * 34].rearrange("c (h w) -> c h w", h=34))
    nc.sync.dma_start(out=outb, in_=zsrc[0:65, 0 : 34 * 34].rearrange("c (h w) -> c h w", h=34))
    nc.sync.dma_start(out=a0, in_=zsrc[0:65, 0 : 44 * 44].rearrange("c (h w) -> c h w", h=44))

    # ---------------- stage A: 1x1 conv + mean + max ----------------
    p66 = ps.tile([66, HW], F32, tag="s")
    for h in (0, 1):
        for k in range(7):
            _mm(
                nc,
                p66[:, h * 512 : (h + 1) * 512],
                wmm[:, k, :],
                xin[:, k, h * 512 : (h + 1) * 512],
                start=(k == 0),
                stop=(k == 6),
            )
    p66v = p66.rearrange("c (h w) -> c h w", h=32)
    nc.scalar.add(xs[:, 3:35, 3:35], p66v, ctab66[:, 59:60])
    nc.scalar.add(x64b, p66v[0:65], b65_ap)

    # pad2 = channel max; all partitions of prd hold the max, add row 65 masked
    nc.vector.tensor_max(acc, xin[:, 0, :], xin[:, 1, :])
    for k in range(2, 7):
        nc.vector.tensor_max(acc, acc, xin[:, k, :])
    nc.gpsimd.partition_all_reduce(prd, acc, 128, bass_isa.ReduceOp.max)
    nc.vector.scalar_tensor_tensor(
        xs[64:66, 3:35, 3:35],
        prd[64:66].rearrange("p (h w) -> p h w", h=32),
        mask2_ap,
        xs[64:66, 3:35, 3:35],
        OP.mult,
        OP.add,
    )

    # ---------------- stage B: branch depthwise convs ----------------
    _dw_stage(
        nc, sb, ps, "br", BR_TAPS, SPLIT["branch"],
        lambda dy, dx: xs[:, 3 + dy : 35 + dy, 3 + dx : 35 + dx],
        xf[:, 1:33, 1:33], 66, wt_br,
        lambda i: diag66[:, i * 66 : (i + 1) * 66], b66_ap,
    )

    # ---------------- stage C: Wpe + identity ----------------
    xf2a = xf2[:, 0:1024].rearrange("c (h w) -> c h w", h=32)
    _dw_stage(
        nc, sb, ps, "pe", PE_TAPS9, SPLIT["wpe"],
        lambda dy, dx: xf[:, 1 + dy : 33 + dy, 1 + dx : 33 + dx],
        xf2a, 66, wt_pe,
        lambda i: diag66[:, (n_br + i) * 66 : (n_br + i + 1) * 66],
        bpe_ap,
    )

    # ---------------- stage D: LayerNorm ----------------
    nc.scalar.square(xf2[:, 1024:2048], xf2[:, 0:1024])
    S = psS.tile([1, 2048], F32)
    for q in range(4):
        _mm(nc, S[:, q * 512 : (q + 1) * 512], ones66, xf2[:, q * 512 : (q + 1) * 512])
    nc.scalar.copy(ms, S)
    nc.scalar.square(msq, ms[:, 0:1024])
    nc.vector.scalar_tensor_tensor(var, msq, -1.0, ms[:, 1024:2048], OP.mult, OP.add)
    nc.scalar.activation(stdv, var, AF.Sqrt, bias=epsv, scale=1.0)
    nc.vector.reciprocal(rmt[:, 0:1024], stdv)
    nc.vector.tensor_mul(rmt[:, 1024:2048], ms[:, 0:1024], rmt[:, 0:1024])
    nc.gpsimd.partition_broadcast(bcast, rmt, channels=66)
    nc.vector.tensor_mul(u66, xf2[:, 0:1024], bcast[:, 0:1024])
    nc.vector.tensor_sub(u66, u66, bcast[:, 1024:2048])
    nc.vector.tensor_scalar(
        lnp[:, 1:33, 1:33], u66.rearrange("c (h w) -> c h w", h=32), g_ap, lb_ap, OP.mult, OP.add
    )

    # ---------------- stage E: Wlast conv + gelu + residual ----------------
    pl = ps.tile([65, HW], F32, tag="s")
    for t, (dy, dx) in enumerate(PE_TAPS9):
        for h in (0, 1):
            _mm(
                nc,
                pl[:, h * 512 : (h + 1) * 512],
                wl[:, t, :],
                lnp[:, 1 + dy + h * 16 : 17 + dy + h * 16, 1 + dx : 33 + dx],
                start=(t == 0),
                stop=(t == 8),
            )
    nc.scalar.activation(
        outb[:, 1:33, 1:33], pl.rearrange("c (h w) -> c h w", h=32), AF.Gelu,
        bias=blast_ap, scale=1.0,
    )
    nc.gpsimd.tensor_add(outb[:, 1:33, 1:33], outb[:, 1:33, 1:33], x64b)

    # ---------------- stage F/G: aux depthwise convs ----------------
    _dw_stage(
        nc, sb, ps, "a0", PE_TAPS9, SPLIT["a0"],
        lambda dy, dx: outb[:, 1 + dy : 33 + dy, 1 + dx : 33 + dx],
        a0[:, 6:38, 6:38], 65, wt_a0,
        lambda i: diag65[:, i * 65 : (i + 1) * 65], None,
    )
    _dw_stage(
        nc, sb, ps, "was", WAS_TAPS, SPLIT["was"],
        lambda dy, dx: a0[:, 6 + dy : 38 + dy, 6 + dx : 38 + dx],
        w5t, 65, wt_was,
        lambda i: diag65[:, (n_a0 + i) * 65 : (n_a0 + i + 1) * 65], None,
    )

    # ---------------- stage H: 1x1 -> 6ch, transposed per-channel images ----------------
    ftp = ps.tile([32, 6, 32], F32, tag="s")
    for y in range(32):
        nc.tensor.matmul(ftp[:, :, y : y + 1], w5t[:, y, :], woutT_ap, start=True, stop=True)
    nc.scalar.copy(fts, ftp)

    # ---------------- stage I: column expansion T_c = ft_c @ A^T ----------------
    for c in range(6):
        Tp = ps.tile([32, 1024], F32, tag="s", name=f"Tp{c}")
        for h in (0, 1):
            _mm(nc, Tp[:, h * 512 : (h + 1) * 512], fts[:, c, :], amat[:, h * 512 : (h + 1) * 512])
        if c % 2 == 0:
            nc.vector.tensor_copy(Ts[:, c * 1024 : (c + 1) * 1024], Tp)
        else:
            nc.scalar.copy(Ts[:, c * 1024 : (c + 1) * 1024], Tp)

    # ---------------- stage J: row expansion + stream out ----------------
    for c in range(6):
        so = None
        for m in range(8):
            if m % 4 == 0:
                so = sbD.tile([128, 4, 1024], F32, tag="so", name=f"so_{c}_{m // 4}")
            po = ps.tile([128, 1024], F32, tag="s", name=f"po_{c}_{m}")
            for h in (0, 1):
                _mm(
                    nc,
                    po[:, h * 512 : (h + 1) * 512],
                    amat[:, m * 128 : (m + 1) * 128],
                    Ts[:, c * 1024 + h * 512 : c * 1024 + (h + 1) * 512],
                )
            if m % 2 == 0:
                nc.vector.tensor_copy(so[:, m % 4, :], po)
            else:
                nc.scalar.copy(so[:, m % 4, :], po)
            if m % 4 == 3:
                g = m // 4
                dst = out_ap[c, g * 512 : (g + 1) * 512, :].rearrange(
                    "(j p) e -> p j e", p=128
                )
                nc.sync.dma_start(out=dst, in_=so)

    ctx.close()


def build_program():
    nc = bacc.Bacc(
        "TRN2",
        target_bir_lowering=False,
        debug=False,
        enable_asserts=False,
        num_devices=NCORES,
    )
    ins = {
        "x": nc.dram_tensor("x", [CIN, 32, 32], RD, kind="ExternalInput").ap(),
        "wmm": nc.dram_tensor("wmm", [128, 7, 66], RD, kind="ExternalInput").ap(),
        "ctab66": nc.dram_tensor("ctab66", [66, 65], F32, kind="ExternalInput").ap(),
        "ctab65": nc.dram_tensor("ctab65", [65, 41], F32, kind="ExternalInput").ap(),
        "wl": nc.dram_tensor("wl", [66, 9, 65], RD, kind="ExternalInput").ap(),
        "diag66": nc.dram_tensor(
            "diag66", [66, (SPLIT["branch"][0] + SPLIT["wpe"][0]) * 66], RD, kind="ExternalInput"
        ).ap(),
        "diag65": nc.dram_tensor(
            "diag65", [65, (SPLIT["a0"][0] + SPLIT["was"][0]) * 65], RD, kind="ExternalInput"
        ).ap(),
        "amat": nc.dram_tensor("amat", [32, 1024], RD, kind="ExternalInput").ap(),
        "zz": nc.dram_tensor("zz", [128, 1936], RD, kind="ExternalInput").ap(),
        "negf": nc.dram_tensor("negf", [32, 1024], RD, kind="ExternalInput").ap(),
    }
    out_ap = nc.dram_tensor("out", [NCH_OUT, HO, HO], F32, kind="ExternalOutput").ap()
    with tile.TileContext(nc) as tc:
        build_kernel_body(tc, out_ap, ins)
    nc.compile()
    return nc


_PROGRAM = None
last_results = None


def _ensure_ntff_hook():
    """Register the axon NTFF profile hook if the agent image lacks
    antenv.axon_hooks (tracing degrades silently otherwise)."""
    try:
        import antenv.axon_hooks  # noqa: F401

        return
    except ImportError:
        pass
    try:
        import sys as _sys
        import types

        import antenv
        from trn_agent_boot.trn_boot import _ntff_profile_via_ctypes

        mod = types.ModuleType("antenv.axon_hooks")
        _state = {"h": None}
        mod.set_axon_ntff_profile_hook = lambda h: _state.__setitem__("h", h)
        mod.get_axon_ntff_profile_hook = lambda: _state["h"]
        antenv.axon_hooks = mod
        _sys.modules["antenv.axon_hooks"] = mod
        mod.set_axon_ntff_profile_hook(
            _ntff_profile_via_ctypes("/opt/axon/libaxon_pjrt.so")
        )
    except Exception:
        pass


def kernel(**inputs) -> np.ndarray:
    global _PROGRAM, last_results
    x = np.ascontiguousarray(np.asarray(inputs["x"], np.float32))
    assert x.shape == (8, CIN, 32, 32)
    assert int(inputs["h"]) == HO and int(inputs["w"]) == HO
    consts = host_consts(inputs)

    if _PROGRAM is None:
        _PROGRAM = build_program()
    nc = _PROGRAM

    in_maps = [dict(consts, x=np.ascontiguousarray(x[b])) for b in range(NCORES)]
    trace = os.environ.get("BASS_TRACE_RUN", "0") == "1"
    if trace:
        _ensure_ntff_hook()
    last_results = run_bass_kernel_spmd(
        nc, in_maps, core_ids=list(range(NCORES)), trace=trace
    )
    return np.stack([r["out"] for r in last_results.results], axis=0)


if __name__ == "__main__":
    rng = np.random.default_rng(0)
    import reference

    inp = reference.setup_inputs()
    out = kernel(**{k: np.asarray(v) if hasattr(v, "shape") else v for k, v in inp.items()})
    print("kernel out", out.shape, out.dtype)
